# revision 21
# baseline (speedup 1.0000x reference)
"""CAM (channel-attention) kernel for Trainium2, data-parallel over batch on 8 cores.

Reference computation (per sample b):
    avg[c] = mean over spatial of x[b, c, :, :]
    mx[c]  = max  over spatial of x[b, c, :, :]
    gate   = sigmoid(W2 @ relu(W1 @ avg) + W2 @ relu(W1 @ mx))
    y[b]   = x[b] * gate[:, None, None]

Design (per core, 4 samples; memory-roofline bound at ~360 GB/s/core):
  - Each sample is SBUF-resident as [128 partitions, 4 channel-groups, 3136]
    (channel c = ci*128 + p): x is read from HBM once and written back once,
    the minimum possible traffic (~51 MB/core -> ~140 us at roofline).
  - Input DMAs ride the SP HW-DGE ring, output DMAs the ACT HW-DGE ring
    (hardware descriptor generation; SWDGE's Q7 descriptor build would
    serialize ~4.8 us per 1.6 MB transfer and become the bottleneck).
  - The per-channel mean comes free on ScalarE: activation(Copy, scale=1/S)
    with accum_out, while VectorE does reduce_max. The scratch `out` of that
    activation is a small rotating dummy tile.
  - The tiny shared MLP runs on TensorE with host-pretransposed weights.
    relu(W1@mean) = relu(W1@sum)/S (positive homogeneity) and W2 is linear,
    so both branches merge into one [32,1] vector before layer 2:
        layer1: psum[32,2] += w1t[:,ci,:].T @ [mean | max], ci=0..3
        hsum = relu(psum)[:,0] + relu(psum)[:,1]
        layer2 per ci: p2[:,ci] = w2t[:,ci*128:].T @ hsum ; gate = sigmoid(p2)
  - Gating multiply on VectorE tensor_scalar (2x fp32 SBUF mode) into fresh
    output tiles, then streamed out per channel-group.
  - A zeroing matmul opens each PSUM accumulation group and warm-up matmuls
    touch the weight tiles once: every hot instruction then needs at most
    one semaphore wait, minimizing Bacc's EventSemaphore splitting (this
    toolchain allows exactly ONE wait slot per hardware instruction).
  - Built on Bacc: nc.compile() legalizes any remaining multi-wait
    instructions into EventSemaphore prefixes.
"""

import numpy as np

import concourse.bacc as bacc
import concourse.bass as bass
import concourse.tile as tile
from concourse import mybir
from concourse.bass_utils import run_bass_kernel_spmd

N_CORES = 8
B = 32
C = 512
S = 56 * 56  # 3136
BPC = B // N_CORES  # samples per core
P = 128
CI = C // P  # channel groups of 128
HID = 32

F32 = mybir.dt.float32
AF = mybir.ActivationFunctionType

LAST_RESULTS = None  # BassKernelResults of the most recent run (for test harness)
_NC_CACHE = None


def _build_bass():
    nc = bacc.Bacc()
    x = nc.dram_tensor("x", (BPC, CI, P, S), F32, kind="ExternalInput")
    w1t = nc.dram_tensor("w1t", (P, CI, HID), F32, kind="ExternalInput")
    w2t = nc.dram_tensor("w2t", (HID, C), F32, kind="ExternalInput")
    y = nc.dram_tensor("y", (BPC, CI, P, S), F32, kind="ExternalOutput")

    with tile.TileContext(nc) as tc:
        with (
            tc.tile_pool(name="xp", bufs=2) as xp,
            tc.tile_pool(name="yp", bufs=4) as yp,
            tc.tile_pool(name="consts", bufs=1) as consts,
            tc.tile_pool(name="small", bufs=4) as small,
            tc.tile_pool(name="dump", bufs=2) as dump,
            tc.tile_pool(name="ps1", bufs=4, space=bass.MemorySpace.PSUM) as ps1,
            tc.tile_pool(name="ps2", bufs=4, space=bass.MemorySpace.PSUM) as ps2,
        ):
            w1t_sb = consts.tile([P, CI, HID], F32)
            nc.sync.dma_start(out=w1t_sb[:], in_=w1t[:])
            w2t_sb = consts.tile([HID, C], F32)
            nc.sync.dma_start(out=w2t_sb[:], in_=w2t[:])
            zeros = consts.tile([P, CI], F32)
            nc.vector.memset(zeros[:], 0.0)

            # PE observes the two weight-DMA semaphores here, once.
            pw = ps1.tile([HID, 2], F32, tag="p1", name="pw")
            nc.tensor.matmul(pw[:, 0:1], w1t_sb[:, 0, :], w1t_sb[:, 0, 0:1])
            pw2 = ps2.tile([P, CI], F32, tag="p2", name="pw2")
            nc.tensor.matmul(pw2[:, 0:1], w2t_sb[:, 0:P], w2t_sb[:, 0:1])

            for b in range(BPC):
                xt = xp.tile([P, CI, S], F32, tag="xt", name=f"xt{b}")
                stats = small.tile([P, CI, 2], F32, tag="stats", name=f"st{b}")
                for ci in range(CI):
                    nc.sync.dma_start(out=xt[:, ci, :], in_=x[b, ci])
                for ci in range(CI):
                    # ScalarE: accum_out = sum(x/S) = mean; dmy is scratch
                    dmy = dump.tile([P, S], F32, tag="dmy", name=f"dmy{b}_{ci}")
                    nc.scalar.activation(
                        out=dmy[:],
                        in_=xt[:, ci, :],
                        func=AF.Copy,
                        scale=1.0 / S,
                        accum_out=stats[:, ci, 0:1],
                    )
                    nc.vector.reduce_max(
                        out=stats[:, ci, 1:2],
                        in_=xt[:, ci, :],
                        axis=mybir.AxisListType.X,
                    )

                # layer 1: psum [32, 2] = sum_ci W1[:, ci-block] @ [mean | max]
                p1 = ps1.tile([HID, 2], F32, tag="p1", name=f"p1_{b}")
                nc.tensor.matmul(
                    p1[:], w1t_sb[:, 0, :], zeros[:, 0:2], start=True, stop=False
                )
                for ci in range(CI):
                    nc.tensor.matmul(
                        p1[:],
                        w1t_sb[:, ci, :],
                        stats[:, ci, :],
                        start=False,
                        stop=(ci == CI - 1),
                    )
                h = small.tile([HID, 2], F32, tag="h", name=f"h{b}")
                nc.scalar.activation(out=h[:], in_=p1[:], func=AF.Relu)
                # W2 is linear: merge branches before layer 2
                hsum = small.tile([HID, 1], F32, tag="hsum", name=f"hs{b}")
                nc.vector.tensor_scalar(
                    out=hsum[:],
                    in0=h[:, 0:1],
                    scalar1=1.0,
                    scalar2=h[:, 1:2],
                    op0=mybir.AluOpType.mult,
                    op1=mybir.AluOpType.add,
                )

                # layer 2: four matmuls into disjoint columns of one PSUM bank
                p2 = ps2.tile([P, CI], F32, tag="p2", name=f"p2_{b}")
                nc.tensor.matmul(
                    p2[:],
                    w2t_sb[:, 0:P],
                    zeros[:HID, 0:CI],
                    start=True,
                    stop=False,
                    skip_group_check=True,
                )
                for ci in range(CI):
                    nc.tensor.matmul(
                        p2[:, ci : ci + 1],
                        w2t_sb[:, ci * P : (ci + 1) * P],
                        hsum[:],
                        start=False,
                        stop=(ci == CI - 1),
                        skip_group_check=True,
                    )
                g = small.tile([P, CI], F32, tag="g", name=f"g{b}")
                nc.scalar.activation(out=g[:], in_=p2[:], func=AF.Sigmoid)
                # single-producer copy: the muls wait on DVE only
                g2 = small.tile([P, CI], F32, tag="g2", name=f"g2_{b}")
                nc.vector.tensor_copy(out=g2[:], in_=g[:])

                for ci in range(CI):
                    yt = yp.tile([P, S], F32, tag="yt", name=f"yt{b}_{ci}")
                    nc.vector.tensor_scalar_mul(
                        out=yt[:],
                        in0=xt[:, ci, :],
                        scalar1=g2[:, ci : ci + 1],
                    )
                    nc.scalar.dma_start(out=y[b, ci], in_=yt[:])
    nc.compile()
    return nc


def kernel(x, w1, w2, _trace=False, **_trace_kwargs):
    global LAST_RESULTS, _NC_CACHE
    x = np.ascontiguousarray(np.asarray(x, dtype=np.float32))
    w1 = np.asarray(w1, dtype=np.float32)  # [HID, C]
    w2 = np.asarray(w2, dtype=np.float32)  # [C, HID]

    # SBUF layouts, pretransposed on host
    w1t = np.ascontiguousarray(
        w1.T.reshape(CI, P, HID).transpose(1, 0, 2)
    )  # [P, CI, HID]; w1t[p, ci, h] = w1[h, ci*128+p]
    w2t = np.ascontiguousarray(w2.T)  # [HID, C]

    if _NC_CACHE is None:
        _NC_CACHE = _build_bass()
    nc = _NC_CACHE

    in_maps = []
    for core in range(N_CORES):
        shard = x[core * BPC : (core + 1) * BPC].reshape(BPC, CI, P, S)
        in_maps.append({"x": np.ascontiguousarray(shard), "w1t": w1t, "w2t": w2t})

    LAST_RESULTS = run_bass_kernel_spmd(
        nc,
        in_maps,
        core_ids=list(range(N_CORES)),
        trace=_trace,
        **_trace_kwargs,
    )
    out = np.concatenate(
        [r["y"].reshape(BPC, C, 56, 56) for r in LAST_RESULTS.results], axis=0
    )
    return out


# revision 23
# speedup vs baseline: 1.2400x; 1.2400x over previous
"""CAM (channel-attention) kernel for Trainium2, data-parallel over batch on 8 cores.

Reference computation (per sample b):
    avg[c] = mean over spatial of x[b, c, :, :]
    mx[c]  = max  over spatial of x[b, c, :, :]
    gate   = sigmoid(W2 @ relu(W1 @ avg) + W2 @ relu(W1 @ mx))
    y[b]   = x[b] * gate[:, None, None]

Design (per core, 4 samples; memory-roofline bound at ~360 GB/s/core):
  - Each sample is SBUF-resident as [128 partitions, 4 channel-groups, 3136]
    (channel c = ci*128 + p): x is read from HBM once and written back once,
    the minimum possible traffic (~51 MB/core -> ~140 us at roofline).
  - Input DMAs ride the SP HW-DGE ring, output DMAs the Pool SWDGE ring:
    a DMA occupies its issuing ring for the whole transfer (~4.8 us per
    1.6 MB), so input and output streams must live on different rings, and
    the ACT ring is kept free for the mean-reduction activations.
  - The per-channel mean comes free on ScalarE: activation(Copy, scale=1/S)
    with accum_out, while VectorE does reduce_max. The scratch `out` of that
    activation is a small rotating dummy tile.
  - The tiny shared MLP runs on TensorE with host-pretransposed weights.
    relu(W1@mean) = relu(W1@sum)/S (positive homogeneity) and W2 is linear,
    so both branches merge into one [32,1] vector before layer 2:
        layer1: psum[32,2] += w1t[:,ci,:].T @ [mean | max], ci=0..3
        hsum = relu(psum)[:,0] + relu(psum)[:,1]
        layer2 per ci: p2[:,ci] = w2t[:,ci*128:].T @ hsum ; gate = sigmoid(p2)
  - Gating multiply on VectorE tensor_scalar (2x fp32 SBUF mode) into fresh
    output tiles, then streamed out per channel-group.
  - A zeroing matmul opens each PSUM accumulation group and warm-up matmuls
    touch the weight tiles once: every hot instruction then needs at most
    one semaphore wait, minimizing Bacc's EventSemaphore splitting (this
    toolchain allows exactly ONE wait slot per hardware instruction).
  - Built on Bacc: nc.compile() legalizes any remaining multi-wait
    instructions into EventSemaphore prefixes.
"""

import numpy as np

import concourse.bacc as bacc
import concourse.bass as bass
import concourse.tile as tile
from concourse import mybir
from concourse.bass_utils import run_bass_kernel_spmd

N_CORES = 8
B = 32
C = 512
S = 56 * 56  # 3136
BPC = B // N_CORES  # samples per core
P = 128
CI = C // P  # channel groups of 128
HID = 32

F32 = mybir.dt.float32
AF = mybir.ActivationFunctionType

LAST_RESULTS = None  # BassKernelResults of the most recent run (for test harness)
_NC_CACHE = None


def _build_bass():
    nc = bacc.Bacc()
    x = nc.dram_tensor("x", (BPC, CI, P, S), F32, kind="ExternalInput")
    w1t = nc.dram_tensor("w1t", (P, CI, HID), F32, kind="ExternalInput")
    w2t = nc.dram_tensor("w2t", (HID, C), F32, kind="ExternalInput")
    y = nc.dram_tensor("y", (BPC, CI, P, S), F32, kind="ExternalOutput")

    with tile.TileContext(nc) as tc:
        with (
            tc.tile_pool(name="xp", bufs=2) as xp,
            tc.tile_pool(name="yp", bufs=4) as yp,
            tc.tile_pool(name="consts", bufs=1) as consts,
            tc.tile_pool(name="small", bufs=4) as small,
            tc.tile_pool(name="dump", bufs=2) as dump,
            tc.tile_pool(name="ps1", bufs=4, space=bass.MemorySpace.PSUM) as ps1,
            tc.tile_pool(name="ps2", bufs=4, space=bass.MemorySpace.PSUM) as ps2,
        ):
            w1t_sb = consts.tile([P, CI, HID], F32)
            nc.sync.dma_start(out=w1t_sb[:], in_=w1t[:])
            w2t_sb = consts.tile([HID, C], F32)
            nc.sync.dma_start(out=w2t_sb[:], in_=w2t[:])
            zeros = consts.tile([P, CI], F32)
            nc.vector.memset(zeros[:], 0.0)

            # PE observes the two weight-DMA semaphores here, once.
            pw = ps1.tile([HID, 2], F32, tag="p1", name="pw")
            nc.tensor.matmul(pw[:, 0:1], w1t_sb[:, 0, :], w1t_sb[:, 0, 0:1])
            pw2 = ps2.tile([P, CI], F32, tag="p2", name="pw2")
            nc.tensor.matmul(pw2[:, 0:1], w2t_sb[:, 0:P], w2t_sb[:, 0:1])

            for b in range(BPC):
                xt = xp.tile([P, CI, S], F32, tag="xt", name=f"xt{b}")
                stats = small.tile([P, CI, 2], F32, tag="stats", name=f"st{b}")
                for ci in range(CI):
                    nc.sync.dma_start(out=xt[:, ci, :], in_=x[b, ci])
                for ci in range(CI):
                    # ScalarE: accum_out = sum(x/S) = mean; dmy is scratch
                    dmy = dump.tile([P, S], F32, tag="dmy", name=f"dmy{b}_{ci}")
                    nc.scalar.activation(
                        out=dmy[:],
                        in_=xt[:, ci, :],
                        func=AF.Copy,
                        scale=1.0 / S,
                        accum_out=stats[:, ci, 0:1],
                    )
                    nc.vector.reduce_max(
                        out=stats[:, ci, 1:2],
                        in_=xt[:, ci, :],
                        axis=mybir.AxisListType.X,
                    )

                # layer 1: psum [32, 2] = sum_ci W1[:, ci-block] @ [mean | max]
                p1 = ps1.tile([HID, 2], F32, tag="p1", name=f"p1_{b}")
                nc.tensor.matmul(
                    p1[:], w1t_sb[:, 0, :], zeros[:, 0:2], start=True, stop=False
                )
                for ci in range(CI):
                    nc.tensor.matmul(
                        p1[:],
                        w1t_sb[:, ci, :],
                        stats[:, ci, :],
                        start=False,
                        stop=(ci == CI - 1),
                    )
                h = small.tile([HID, 2], F32, tag="h", name=f"h{b}")
                nc.scalar.activation(out=h[:], in_=p1[:], func=AF.Relu)
                # W2 is linear: merge branches before layer 2
                hsum = small.tile([HID, 1], F32, tag="hsum", name=f"hs{b}")
                nc.vector.tensor_scalar(
                    out=hsum[:],
                    in0=h[:, 0:1],
                    scalar1=1.0,
                    scalar2=h[:, 1:2],
                    op0=mybir.AluOpType.mult,
                    op1=mybir.AluOpType.add,
                )

                # layer 2: four matmuls into disjoint columns of one PSUM bank
                p2 = ps2.tile([P, CI], F32, tag="p2", name=f"p2_{b}")
                nc.tensor.matmul(
                    p2[:],
                    w2t_sb[:, 0:P],
                    zeros[:HID, 0:CI],
                    start=True,
                    stop=False,
                    skip_group_check=True,
                )
                for ci in range(CI):
                    nc.tensor.matmul(
                        p2[:, ci : ci + 1],
                        w2t_sb[:, ci * P : (ci + 1) * P],
                        hsum[:],
                        start=False,
                        stop=(ci == CI - 1),
                        skip_group_check=True,
                    )
                g = small.tile([P, CI], F32, tag="g", name=f"g{b}")
                nc.scalar.activation(out=g[:], in_=p2[:], func=AF.Sigmoid)
                # single-producer copy: the muls wait on DVE only
                g2 = small.tile([P, CI], F32, tag="g2", name=f"g2_{b}")
                nc.vector.tensor_copy(out=g2[:], in_=g[:])

                for ci in range(CI):
                    yt = yp.tile([P, S], F32, tag="yt", name=f"yt{b}_{ci}")
                    nc.vector.tensor_scalar_mul(
                        out=yt[:],
                        in0=xt[:, ci, :],
                        scalar1=g2[:, ci : ci + 1],
                    )
                    nc.gpsimd.dma_start(out=y[b, ci], in_=yt[:])
    nc.compile()
    return nc


def kernel(x, w1, w2, _trace=False, **_trace_kwargs):
    global LAST_RESULTS, _NC_CACHE
    x = np.ascontiguousarray(np.asarray(x, dtype=np.float32))
    w1 = np.asarray(w1, dtype=np.float32)  # [HID, C]
    w2 = np.asarray(w2, dtype=np.float32)  # [C, HID]

    # SBUF layouts, pretransposed on host
    w1t = np.ascontiguousarray(
        w1.T.reshape(CI, P, HID).transpose(1, 0, 2)
    )  # [P, CI, HID]; w1t[p, ci, h] = w1[h, ci*128+p]
    w2t = np.ascontiguousarray(w2.T)  # [HID, C]

    if _NC_CACHE is None:
        _NC_CACHE = _build_bass()
    nc = _NC_CACHE

    in_maps = []
    for core in range(N_CORES):
        shard = x[core * BPC : (core + 1) * BPC].reshape(BPC, CI, P, S)
        in_maps.append({"x": np.ascontiguousarray(shard), "w1t": w1t, "w2t": w2t})

    LAST_RESULTS = run_bass_kernel_spmd(
        nc,
        in_maps,
        core_ids=list(range(N_CORES)),
        trace=_trace,
        **_trace_kwargs,
    )
    out = np.concatenate(
        [r["y"].reshape(BPC, C, 56, 56) for r in LAST_RESULTS.results], axis=0
    )
    return out
